# revision 1
# baseline (speedup 1.0000x reference)
"""Distributed Trainium2 Bass kernel for 16-head attention.

Reference op: B=2, S=2048, D=1024, H=16 multi-head attention with an
elementwise 0/1 mask, computed as
    out = softmax(mask((q Wq^T)(k Wk^T)^T / sqrt(64))) (v Wv^T) Wo^T

Sharding over 8 NeuronCores: core c handles batch c//4 and head group
c%4 (4 heads = 256 channels). Attention is computed fully locally in a
"dual" layout (scores transposed, [k, q]); the context is exchanged
with one small AllGather per 512-query tile inside each 4-core batch
group, and the output projection is split along the OUTPUT feature dim
(each core holds a 256-column slice of Wo^T), so the host-side unshard
is a pure concatenation.

Compute dtype bf16 (TensorE 1 cyc/row), accumulation f32 in PSUM.
"""

import sys
import types

sys.path.insert(0, "/opt/trn_rl_repo")

import numpy as np
import ml_dtypes

BF16 = ml_dtypes.bfloat16

B = 2
S = 2048
DM = 1024
DL = 256  # d-model slice per core (4 heads)
HL = 4  # heads per core
DK = 64
P = 128
QT_N = 4  # query tiles of 512
QTS = 512
KC = 16  # key chunks of 128
MC = 8  # contraction chunks of 128 over d_model
GROUPS = [[0, 1, 2, 3], [4, 5, 6, 7]]

_cached = {}


def _build():
    import concourse.bass as bass
    import concourse.mybir as mybir
    from concourse import bacc
    from concourse.tile import TileContext

    fp32 = mybir.dt.float32
    bf16 = mybir.dt.bfloat16

    nc = bacc.Bacc(num_devices=8)

    qT = nc.dram_tensor("qT", [DM, S], bf16, kind="ExternalInput")
    kT = nc.dram_tensor("kT", [DM, S], bf16, kind="ExternalInput")
    vT = nc.dram_tensor("vT", [DM, S], bf16, kind="ExternalInput")
    maskT = nc.dram_tensor("maskT", [S, S], bf16, kind="ExternalInput")
    wq = nc.dram_tensor("wq", [DM, DL], bf16, kind="ExternalInput")
    wk = nc.dram_tensor("wk", [DM, DL], bf16, kind="ExternalInput")
    wv = nc.dram_tensor("wv", [DM, DL], bf16, kind="ExternalInput")
    wo = nc.dram_tensor("wo", [DM, DL], bf16, kind="ExternalInput")
    y = nc.dram_tensor("y", [S, DL], fp32, kind="ExternalOutput")

    cc_in = [
        [
            nc.dram_tensor(f"cc_in{t}_{p}", [P, QTS], bf16, kind="Internal")
            for p in range(2)
        ]
        for t in range(QT_N)
    ]
    cc_out = [
        [
            nc.dram_tensor(f"cc_out{t}_{p}", [4 * P, QTS], bf16, kind="Internal")
            for p in range(2)
        ]
        for t in range(QT_N)
    ]

    with TileContext(nc) as tc:
        with (
            tc.tile_pool(name="xT", bufs=9) as xT_pool,
            tc.tile_pool(name="w", bufs=32) as w_pool,
            tc.tile_pool(name="qkt", bufs=2) as qkt_pool,
            tc.tile_pool(name="vext", bufs=16) as vext_pool,
            tc.tile_pool(name="mask", bufs=2) as mask_pool,
            tc.tile_pool(name="attn", bufs=4) as attn_pool,
            tc.tile_pool(name="sm", bufs=3) as sm_pool,
            tc.tile_pool(name="ctxn", bufs=8) as ctxn_pool,
            tc.tile_pool(name="ctxg", bufs=4) as ctxg_pool,
            tc.tile_pool(name="ysb", bufs=2) as y_pool,
            tc.tile_pool(name="ps_big", bufs=2, space="PSUM") as ps_big,
            tc.tile_pool(name="ps_acc", bufs=2, space="PSUM") as ps_acc,
            tc.tile_pool(name="ps_out", bufs=2, space="PSUM") as ps_out,
        ):
            # ---- weight loads -------------------------------------------------
            def load_w(dram):
                tiles = []
                for m in range(MC):
                    t_ = w_pool.tile([P, DL], bf16, tag="w")
                    nc.sync.dma_start(t_[:], dram[P * m : P * (m + 1), :])
                    tiles.append(t_)
                return tiles

            # ---- Q/K projections: out QT/KT [256, 2048] as 2 tiles [128,2048]
            def proj_T(x_dram, w_sb, tag, split_first=False):
                x_sb = []
                for m in range(MC):
                    t_ = xT_pool.tile([P, S], bf16, tag="xT")
                    if split_first:
                        for cq in range(4):
                            nc.sync.dma_start(
                                t_[:, QTS * cq : QTS * (cq + 1)],
                                x_dram[P * m : P * (m + 1), QTS * cq : QTS * (cq + 1)],
                            )
                    else:
                        nc.sync.dma_start(t_[:], x_dram[P * m : P * (m + 1), :])
                    x_sb.append(t_)
                out_tiles = []
                for dt in range(2):
                    ot = qkt_pool.tile([P, S], bf16, tag=tag)
                    for st in range(2):
                        ps = ps_big.tile([P, 1024], fp32, tag="big")
                        for m in range(MC):
                            for sh in range(2):
                                nc.tensor.matmul(
                                    ps[:, QTS * sh : QTS * (sh + 1)],
                                    w_sb[m][:, P * dt : P * (dt + 1)],
                                    x_sb[m][
                                        :,
                                        1024 * st + QTS * sh : 1024 * st + QTS * (sh + 1),
                                    ],
                                    start=(m == 0),
                                    stop=(m == MC - 1),
                                )
                        nc.vector.tensor_copy(
                            ot[:, 1024 * st : 1024 * (st + 1)], ps[:]
                        )
                    out_tiles.append(ot)
                return out_tiles

            wq_sb = load_w(wq)
            QT_sb = proj_T(qT, wq_sb, "QT", split_first=True)
            wk_sb = load_w(wk)
            KT_sb = proj_T(kT, wk_sb, "KT")
            wv_sb = load_w(wv)

            # ---- V projection -> V_ext tiles [128, 4*65] ([V_h | 1] blocks)
            vT_sb = []
            for m in range(MC):
                t_ = xT_pool.tile([P, S], bf16, tag="xT")
                nc.sync.dma_start(t_[:], vT[P * m : P * (m + 1), :])
                vT_sb.append(t_)
            vext = []
            for st in range(KC):
                ps = ps_acc.tile([P, QTS], fp32, tag="acc")
                for m in range(MC):
                    nc.tensor.matmul(
                        ps[:, 0:DL],
                        vT_sb[m][:, P * st : P * (st + 1)],
                        wv_sb[m][:],
                        start=(m == 0),
                        stop=(m == MC - 1),
                    )
                ve = vext_pool.tile([P, HL * (DK + 1)], bf16, tag="vext")
                nc.vector.memset(ve[:], 1.0)
                for h in range(HL):
                    nc.vector.tensor_copy(
                        ve[:, 65 * h : 65 * h + DK],
                        ps[:, DK * h : DK * (h + 1)],
                    )
                vext.append(ve)

            wo_sb = load_w(wo)

            def load_mask(t):
                mt_ = mask_pool.tile(
                    [P, KC * QTS], bf16, tag="mask", name=f"mask{t}"
                )
                src3 = maskT.rearrange("(kc p) q -> p kc q", p=P)[
                    :, :, QTS * t : QTS * (t + 1)
                ]
                dst3 = mt_[:].rearrange("p (kc q) -> p kc q", q=QTS)
                nc.sync.dma_start(dst3, src3)
                return mt_

            mts = {0: load_mask(0)}

            ones_lhs = sm_pool.tile([DK + 1, P], bf16, tag="ones")
            nc.vector.memset(ones_lhs[:], 1.0)

            # ---- attention + exchange + output projection per query tile ----
            # The exchange readback + output projection for query tile t are
            # issued inside tile t+1's block so the AllGather latency hides
            # under the next tile's attention and never head-of-line-blocks
            # an engine queue.
            def do_readback(t, pairs=(0, 1)):
                ctxg = []
                for p in pairs:
                    cg = ctxg_pool.tile(
                        [P, 4 * QTS], bf16, tag="ctxg", name=f"cg{t}_{p}"
                    )
                    src3 = cc_out[t][p].rearrange("(i pp) q -> pp i q", pp=P)
                    dst3 = cg[:].rearrange("pp (i q) -> pp i q", q=QTS)
                    nc.sync.dma_start(dst3, src3)
                    ctxg.append(cg)
                return ctxg

            DCS = [0, 2, 4, 6, 1, 3, 5, 7]

            def outproj_steps(t, ctxg):
                # Generator of small out-proj work units (2 matmuls each) to
                # interleave into the next tile's attention stream, keeping
                # the PE queue stocked with always-ready work.
                state = {}

                def unit(qs, i0):
                    if qs not in state:
                        state[qs] = ps_out.tile(
                            [P, DL], fp32, tag="out", name=f"op{t}_{qs}"
                        )
                    op = state[qs]
                    for i in (i0, i0 + 1):
                        dc = DCS[i]
                        src = ctxg[dc % 2][
                            :,
                            QTS * (dc // 2) + P * qs : QTS * (dc // 2)
                            + P * (qs + 1),
                        ]
                        nc.tensor.matmul(
                            op[:],
                            src,
                            wo_sb[dc][:],
                            start=(i == 0),
                            stop=(i == MC - 1),
                        )
                    if i0 + 2 == MC:
                        ys = y_pool.tile(
                            [P, DL], fp32, tag="ysb", name=f"ys{t}_{qs}"
                        )
                        nc.vector.tensor_copy(ys[:], op[:])
                        r = QTS * t + P * qs
                        nc.sync.dma_start(y[r : r + P, :], ys[:])

                for qs in range(4):
                    for i0 in range(0, MC, 2):
                        yield lambda qs=qs, i0=i0: unit(qs, i0)

            def do_outproj(t, ctxg, qs_list=(0, 1, 2, 3)):
                steps = list(outproj_steps(t, ctxg))
                for st_ in steps:
                    st_()

            # ---- flat slot pipeline over (qtile, pair, group) ----------------
            # 64 scores/exp/mask slots; ctx accumulation trails by 3 slots and
            # flows continuously across pair and qtile boundaries so the PE
            # stream never thins out (HAM stays warm). attnT tiles are rolling
            # 8-chunk buffers.
            ATD = 8
            at_store = {}
            cp_store = {}
            rolling_cols = ATD * QTS

            def emit_scores(u, grp):
                t, pair = divmod(u, 2)
                if grp == 0:
                    at_store[u] = {
                        h01: attn_pool.tile(
                            [P, rolling_cols], bf16, tag="attn",
                            name=f"at{u}_{h01}",
                        )
                        for h01 in range(2)
                    }
                    if pair == 0 and t + 1 < QT_N:
                        mts[t + 1] = load_mask(t + 1)
                at = at_store[u]
                mt = mts[t]
                sp = {}
                for h01 in range(2):
                    sp[h01] = ps_big.tile(
                        [P, 1024], fp32, tag="big", name=f"sp{u}_{grp}_{h01}"
                    )
                for j in range(2):
                    kc = 2 * grp + j
                    for h01 in range(2):
                        r0 = DK * h01
                        nc.tensor.matmul(
                            sp[h01][:, QTS * j : QTS * (j + 1)],
                            KT_sb[pair][r0 : r0 + DK, P * kc : P * (kc + 1)],
                            QT_sb[pair][r0 : r0 + DK, QTS * t : QTS * (t + 1)],
                            start=True,
                            stop=True,
                            tile_position=(r0, 0),
                        )
                roff = (2 * grp % ATD) * QTS
                rsl = slice(roff, roff + 1024)
                gsl = slice(1024 * grp, 1024 * (grp + 1))
                for h01 in range(2):
                    nc.scalar.activation(
                        at[h01][:, rsl],
                        sp[h01][:],
                        mybir.ActivationFunctionType.Exp,
                    )
                    nc.vector.tensor_mul(at[h01][:, rsl], at[h01][:, rsl], mt[:, gsl])

            def emit_ctx(u, grp):
                t, pair = divmod(u, 2)
                if grp == 0:
                    cp_store[u] = {
                        h01: ps_acc.tile(
                            [P, QTS], fp32, tag="acc", name=f"cp{u}_{h01}"
                        )
                        for h01 in range(2)
                    }
                at = at_store[u]
                cp = cp_store[u]
                for j in range(2):
                    kc = 2 * grp + j
                    roff = (kc % ATD) * QTS
                    for h01 in range(2):
                        h = 2 * pair + h01
                        nc.tensor.matmul(
                            cp[h01][0 : DK + 1, :],
                            vext[kc][:, 65 * h : 65 * h + DK + 1],
                            at[h01][:, roff : roff + QTS],
                            start=(kc == 0),
                            stop=(kc == KC - 1),
                        )

            def emit_norm(u):
                t, pair = divmod(u, 2)
                cp = cp_store[u]
                for h01 in range(2):
                    srow = sm_pool.tile(
                        [DK + 1, QTS], bf16, tag="srow", name=f"srow{u}_{h01}"
                    )
                    nc.vector.tensor_copy(
                        srow[DK : DK + 1, :], cp[h01][DK : DK + 1, :]
                    )
                    bc = ps_out.tile(
                        [P, QTS], fp32, tag="out", name=f"bc{u}_{h01}"
                    )
                    nc.tensor.matmul(
                        bc[:],
                        ones_lhs[DK : DK + 1, :],
                        srow[DK : DK + 1, :],
                        start=True,
                        stop=True,
                        tile_position=(DK, 0),
                    )
                    recipb = sm_pool.tile(
                        [P, QTS], fp32, tag="recipb", name=f"recipb{u}_{h01}"
                    )
                    nc.vector.reciprocal_approx_fast(out=recipb[:], in_=bc[:])
                    cn = ctxn_pool.tile(
                        [DK, QTS], bf16, tag="ctxn", name=f"cn{u}_{h01}"
                    )
                    nc.vector.tensor_mul(
                        cn[:], cp[h01][0:DK, :], recipb[0:DK, :]
                    )
                    nc.sync.dma_start(
                        cc_in[t][pair][DK * h01 : DK * (h01 + 1), :], cn[:]
                    )
                nc.gpsimd.collective_compute(
                    "AllGather",
                    mybir.AluOpType.bypass,
                    replica_groups=GROUPS,
                    ins=[cc_in[t][pair][:]],
                    outs=[cc_out[t][pair][:]],
                )
                del cp_store[u], at_store[u]

            op_steps = []
            NSLOT = 8 * 2 * QT_N
            ctx_done = 0  # flat index of next ctx slot to emit

            def emit_ctx_flat(lag):
                ul, gl = divmod(lag, 8)
                emit_ctx(ul, gl)
                if gl == 7:
                    emit_norm(ul)
                    tl, pl = divmod(ul, 2)
                    if pl == 1 and tl < QT_N - 1:
                        ctxg_t = do_readback(tl)
                        op_steps.extend(outproj_steps(tl, ctxg_t))

            for i in range(NSLOT):
                u, grp = divmod(i, 8)
                emit_scores(u, grp)
                for _ in range(min(2, len(op_steps))):
                    op_steps.pop(0)()
                # trail by 3 slots; in the final unit converge to lag 1 so the
                # last exchanges issue as early as possible
                target = i - 3 if i < NSLOT - 8 else i - 1
                while ctx_done <= target and ctx_done < NSLOT:
                    emit_ctx_flat(ctx_done)
                    ctx_done += 1
            while ctx_done < NSLOT:
                emit_ctx_flat(ctx_done)
                ctx_done += 1
            ctxg_last = do_readback(QT_N - 1)
            for st_ in op_steps:
                st_()
            do_outproj(QT_N - 1, ctxg_last)

    nc.compile()
    return nc


def _get_nc():
    if "nc" not in _cached:
        _cached["nc"] = _build()
    return _cached["nc"]


def _shard_inputs(q, k, v, mask, w_q, w_k, w_v, w_o):
    in_maps = []
    scale = 1.0 / np.sqrt(DK)
    wqT = (w_q.astype(np.float64) * scale).astype(np.float32).T  # [DM, DM]
    wkT = w_k.T
    wvT = w_v.T
    woT = w_o.T
    for c in range(8):
        b, g = c // 4, c % 4
        sl = slice(DL * g, DL * (g + 1))
        in_maps.append(
            {
                "qT": np.ascontiguousarray(q[b].T).astype(BF16),
                "kT": np.ascontiguousarray(k[b].T).astype(BF16),
                "vT": np.ascontiguousarray(v[b].T).astype(BF16),
                "maskT": np.ascontiguousarray(mask[b].T).astype(BF16),
                "wq": np.ascontiguousarray(wqT[:, sl]).astype(BF16),
                "wk": np.ascontiguousarray(wkT[:, sl]).astype(BF16),
                "wv": np.ascontiguousarray(wvT[:, sl]).astype(BF16),
                "wo": np.ascontiguousarray(woT[:, sl]).astype(BF16),
            }
        )
    return in_maps


def kernel(q, k, v, mask, w_q, w_k, w_v, w_o, _trace=False, _tmpdir=None):
    from concourse import bass_utils

    nc = _get_nc()
    in_maps = _shard_inputs(q, k, v, mask, w_q, w_k, w_v, w_o)
    res = bass_utils.run_bass_kernel_spmd(
        nc,
        in_maps,
        core_ids=list(range(8)),
        trace=_trace,
        tmpdir=_tmpdir,
    )
    out = np.empty((B, S, DM), dtype=np.float32)
    for c in range(8):
        b, g = c // 4, c % 4
        out[b, :, DL * g : DL * (g + 1)] = res.results[c]["y"]
    if _trace:
        _cached["last_exec_time_ns"] = res.exec_time_ns
        _cached["last_results"] = res
    return out



# revision 7
# speedup vs baseline: 1.0480x; 1.0480x over previous
"""Distributed Trainium2 Bass kernel for 16-head attention.

Reference op: B=2, S=2048, D=1024, H=16 multi-head attention with an
elementwise 0/1 mask, computed as
    out = softmax(mask((q Wq^T)(k Wk^T)^T / sqrt(64))) (v Wv^T) Wo^T

Sharding over 8 NeuronCores: core c handles batch c//4 and head group
c%4 (4 heads = 256 channels). Attention is computed fully locally in a
"dual" layout (scores transposed, [k, q]); the context is exchanged
with one small AllGather per (qtile, head-pair) inside each 4-core
batch group, and the output projection is split along the OUTPUT
feature dim (each core holds a 256-column slice of Wo^T), so the
host-side unshard is a pure concatenation.

v2 structure: slots are ordered pair-outer (u = 4*pair + t) so that
the pair-1 Q/K projections become mid-loop PE filler; projections are
emitted as per-(pair, 512-block) units on a per-slot schedule so the
PE never idles long enough for the HAM clock gate to re-throttle.
Mask arrives host-pre-tiled so each tile is one contiguous DMA.

Compute dtype bf16 (TensorE 1 cyc/row), accumulation f32 in PSUM.
"""

import sys

sys.path.insert(0, "/opt/trn_rl_repo")

import numpy as np
import ml_dtypes

BF16 = ml_dtypes.bfloat16

B = 2
S = 2048
DM = 1024
DL = 256  # d-model slice per core (4 heads)
HL = 4  # heads per core
DK = 64
P = 128
QT_N = 4  # query tiles of 512
QTS = 512
KC = 16  # key chunks of 128
MC = 8  # contraction chunks of 128 over d_model
GROUPS = [[0, 1, 2, 3], [4, 5, 6, 7]]

# slot order: u = 4*pair + t
U_LIST = [(0, 0), (0, 1), (0, 2), (0, 3), (1, 0), (1, 1), (1, 2), (1, 3)]

_cached = {}


def _build():
    import concourse.bass as bass
    import concourse.mybir as mybir
    from concourse import bacc
    from concourse.tile import TileContext

    fp32 = mybir.dt.float32
    bf16 = mybir.dt.bfloat16

    nc = bacc.Bacc(num_devices=8)

    qT = nc.dram_tensor("qT", [DM, S], bf16, kind="ExternalInput")
    kT = nc.dram_tensor("kT", [DM, S], bf16, kind="ExternalInput")
    vT = nc.dram_tensor("vT", [DM, S], bf16, kind="ExternalInput")
    # mask pre-tiled on host: maskt[128*t + p, kc*512 + q] = mask[b][512t+q, 128kc+p]
    maskt = nc.dram_tensor("maskt", [QT_N * P, KC * QTS], bf16, kind="ExternalInput")
    wq = nc.dram_tensor("wq", [DM, DL], bf16, kind="ExternalInput")
    wk = nc.dram_tensor("wk", [DM, DL], bf16, kind="ExternalInput")
    wv = nc.dram_tensor("wv", [DM, DL], bf16, kind="ExternalInput")
    wo = nc.dram_tensor("wo", [DM, DL], bf16, kind="ExternalInput")
    y = nc.dram_tensor("y", [S, DL], fp32, kind="ExternalOutput")

    cc_in = [
        [
            nc.dram_tensor(f"cc_in{t}_{p}", [P, QTS], bf16, kind="Internal")
            for p in range(2)
        ]
        for t in range(QT_N)
    ]
    cc_out = [
        [
            nc.dram_tensor(f"cc_out{t}_{p}", [4 * P, QTS], bf16, kind="Internal")
            for p in range(2)
        ]
        for t in range(QT_N)
    ]

    with TileContext(nc) as tc:
        with (
            tc.tile_pool(name="stage", bufs=16) as stage_pool,  # kT/qT/vT slices
            tc.tile_pool(name="w", bufs=32) as w_pool,
            tc.tile_pool(name="qkt", bufs=2) as qkt_pool,  # QT/KT [128,2048]
            tc.tile_pool(name="vext", bufs=16) as vext_pool,
            tc.tile_pool(name="mask", bufs=4) as mask_pool,
            tc.tile_pool(name="attn", bufs=4) as attn_pool,
            tc.tile_pool(name="sm", bufs=3) as sm_pool,
            tc.tile_pool(name="ctxn", bufs=8) as ctxn_pool,
            tc.tile_pool(name="ctxg", bufs=4) as ctxg_pool,
            tc.tile_pool(name="ysb", bufs=2) as y_pool,
            tc.tile_pool(name="ps_big", bufs=2, space="PSUM") as ps_big,
            tc.tile_pool(name="ps_acc", bufs=2, space="PSUM") as ps_acc,
            tc.tile_pool(name="ps_out", bufs=2, space="PSUM") as ps_out,
        ):
            # ---- weight loads ------------------------------------------------
            def load_w(dram, nm):
                tiles = []
                for m in range(MC):
                    t_ = w_pool.tile([P, DL], bf16, tag="w", name=f"w{nm}{m}")
                    nc.sync.dma_start(t_[:], dram[P * m : P * (m + 1), :])
                    tiles.append(t_)
                return tiles

            # ---- staged input slices ----------------------------------------
            # each is a [128, 512] block of the transposed input
            def load_slice(dram, m, st, nm):
                t_ = stage_pool.tile([P, QTS], bf16, tag="stage", name=f"x{nm}")
                nc.sync.dma_start(
                    t_[:], dram[P * m : P * (m + 1), QTS * st : QTS * (st + 1)]
                )
                return t_

            # ---- projection units -------------------------------------------
            # one unit: out_sb[:, 512st:512(st+1)] = (w slice).T @ x[:, 512 st]
            # 8 matmuls N=512 accumulating in a [128,512] psum + 1 cast.
            def proj_unit(x_slices, w_sb, wcol, out_sb, st, nm):
                ps = ps_out.tile([P, QTS], fp32, tag="out", name=f"pp{nm}")
                for m in range(MC):
                    nc.tensor.matmul(
                        ps[:],
                        w_sb[m][:, P * wcol : P * (wcol + 1)],
                        x_slices[m][:],
                        start=(m == 0),
                        stop=(m == MC - 1),
                    )
                nc.vector.tensor_copy(out_sb[:, QTS * st : QTS * (st + 1)], ps[:])

            # ---- V projection unit -> vext[st] [128, 4*65] ------------------
            # ([V_h | 1] blocks; ones columns memset once per tile)
            def v_unit(vT_slices, soff, wv_sb, st):
                # vT_slices: the [128,512] staged tiles covering keys
                # [512*(st//4) .. ); soff = st % 4 selects the 128-col slice
                ps = ps_out.tile([P, DL], fp32, tag="out", name=f"vp{st}")
                for m in range(MC):
                    nc.tensor.matmul(
                        ps[:],
                        vT_slices[m][:, P * soff : P * (soff + 1)],
                        wv_sb[m][:],
                        start=(m == 0),
                        stop=(m == MC - 1),
                    )
                ve = vext_pool.tile(
                    [P, HL * (DK + 1)], bf16, tag="vext", name=f"ve{st}"
                )
                nc.vector.memset(
                    ve[:].rearrange("p (h d) -> p h d", d=DK + 1)[:, :, DK : DK + 1],
                    1.0,
                )
                dst3 = ve[:].rearrange("p (h d) -> p h d", d=DK + 1)[:, :, 0:DK]
                src3 = ps[:].rearrange("p (h d) -> p h d", d=DK)
                nc.vector.tensor_copy(dst3, src3)
                return ve

            # ---- mask loads (4 pieces per tile, each [128, 2048] contig) ----
            def load_mask(t):
                mt_ = mask_pool.tile(
                    [P, KC * QTS], bf16, tag="mask", name=f"mask{t}"
                )
                return mt_

            def load_mask_piece(mt_, t, piece):
                c0 = 2048 * piece
                nc.sync.dma_start(
                    mt_[:, c0 : c0 + 2048],
                    maskt[P * t : P * (t + 1), c0 : c0 + 2048],
                )

            # =================================================================
            # startup: weights + kT; K proj pair0; qT t0; Q proj (0, t0);
            # mask t0 pieces.
            # =================================================================
            wk_sb = load_w(wk, "k")
            wq_sb = load_w(wq, "q")

            # kT slices [m][st]
            kT_sl = [[None] * 4 for _ in range(MC)]
            for st in range(4):
                for m in range(MC):
                    kT_sl[m][st] = load_slice(kT, m, st, f"k{m}_{st}")

            mts = {0: load_mask(0)}
            load_mask_piece(mts[0], 0, 0)

            KT_sb = [None, None]
            KT_sb[0] = qkt_pool.tile([P, S], bf16, tag="KT", name="KT0")
            for st in range(4):
                proj_unit(
                    [kT_sl[m][st] for m in range(MC)], wk_sb, 0, KT_sb[0], st,
                    f"k0_{st}",
                )

            qT_sl = [[None] * 4 for _ in range(MC)]  # [m][t] (pair0 use)
            for m in range(MC):
                qT_sl[m][0] = load_slice(qT, m, 0, f"q{m}_0")
            load_mask_piece(mts[0], 0, 1)

            QT_sb = [None, None]
            QT_sb[0] = qkt_pool.tile([P, S], bf16, tag="QT", name="QT0")
            proj_unit([qT_sl[m][0] for m in range(MC)], wq_sb, 0, QT_sb[0], 0, "q0_0")
            load_mask_piece(mts[0], 0, 2)
            load_mask_piece(mts[0], 0, 3)

            wv_sb = load_w(wv, "v")

            # =================================================================
            # filler schedule: slot index -> list of closures (emitted there)
            # =================================================================
            SLOTS = 8 * 8
            filler = [[] for _ in range(SLOTS + 1)]
            vext = [None] * KC
            vT_sl = [[None] * 4 for _ in range(MC)]

            def sched(slot, fn):
                filler[min(slot, SLOTS)].append(fn)

            # vT slice DMAs + V units: vext[st] needed at slot 3 + st//2
            def mk_vdma(stq):
                def f():
                    for m in range(MC):
                        vT_sl[m][stq] = load_slice(vT, m, stq, f"v{m}_{stq}")
                return f

            def mk_vunit(st):
                def f():
                    stq, soff = divmod(st, 4)
                    vext[st] = v_unit(
                        [vT_sl[m][stq] for m in range(MC)], soff, wv_sb, st
                    )
                return f

            # stq 0 dma at startup (below), others scheduled
            for m in range(MC):
                vT_sl[m][0] = load_slice(vT, m, 0, f"v{m}_0")
            for st in range(KC):
                if st % 4 == 0 and st > 0:
                    sched(st // 2 - 2, mk_vdma(st // 4))
                sched(st // 2, mk_vunit(st))

            # Q proj (pair0, t1-3): needed at slot 8t
            def mk_qdma(t):
                def f():
                    for m in range(MC):
                        qT_sl[m][t] = load_slice(qT, m, t, f"q{m}_{t}")
                return f

            def mk_qunit(pair, t, nm):
                def f():
                    proj_unit(
                        [qT_sl[m][t] for m in range(MC)], wq_sb, pair,
                        QT_sb[pair], t, nm,
                    )
                return f

            for t in range(1, 4):
                sched(4 * t - 3, mk_qdma(t))
                sched(4 * t - 2, mk_qunit(0, t, f"q0_{t}"))

            # wo weights (needed by first outproj ~ slot 12)
            wo_sb = load_w(wo, "o")

            # K proj pair1 at slots 14-21 (kT slices reloaded), needed slot 32
            kT_sl2 = [[None] * 4 for _ in range(MC)]

            def mk_kdma2(st):
                def f():
                    for m in range(MC):
                        kT_sl2[m][st] = load_slice(kT, m, st, f"k2{m}_{st}")
                return f

            def mk_kunit2(st):
                def f():
                    if KT_sb[1] is None:
                        KT_sb[1] = qkt_pool.tile(
                            [P, S], bf16, tag="KT", name="KT1"
                        )
                    proj_unit(
                        [kT_sl2[m][st] for m in range(MC)], wk_sb, 1,
                        KT_sb[1], st, f"k1_{st}",
                    )
                return f

            # KT_sb[1] allocation must not recycle KT_sb[0] (bufs=2 -> ok)
            for st in range(4):
                sched(12 + 2 * st, mk_kdma2(st))
                sched(14 + 2 * st, mk_kunit2(st))

            # Q proj pair1: needed at slots 32,40,48,56; qT slices reloaded
            qT_sl2 = [[None] * 4 for _ in range(MC)]

            def mk_qdma2(t):
                def f():
                    for m in range(MC):
                        qT_sl2[m][t] = load_slice(qT, m, t, f"q2{m}_{t}")
                return f

            def mk_qunit2(t, nm):
                def f():
                    if QT_sb[1] is None:
                        QT_sb[1] = qkt_pool.tile(
                            [P, S], bf16, tag="QT", name="QT1"
                        )
                    proj_unit(
                        [qT_sl2[m][t] for m in range(MC)], wq_sb, 1,
                        QT_sb[1], t, nm,
                    )
                return f

            for t in range(4):
                sched(23 + 5 * t, mk_qdma2(t))
                sched(25 + 5 * t, mk_qunit2(t, f"q1_{t}"))

            # mask tiles 1-3: tile t needed at slot 8t (pair0), resident after
            def mk_mdma(t, piece):
                def f():
                    if t not in mts:
                        mts[t] = load_mask(t)
                    load_mask_piece(mts[t], t, piece)
                return f

            for t in range(1, 4):
                for piece in range(4):
                    sched(8 * t - 7 + piece, mk_mdma(t, piece))

            # =================================================================
            # attention slot pipeline
            # =================================================================
            ones_lhs = sm_pool.tile([DK + 1, P], bf16, tag="ones")
            nc.vector.memset(ones_lhs[:], 1.0)

            DCS = [0, 2, 4, 6, 1, 3, 5, 7]

            def do_readback(t, pairs=(0, 1)):
                ctxg = []
                for p in pairs:
                    cg = ctxg_pool.tile(
                        [P, 4 * QTS], bf16, tag="ctxg", name=f"cg{t}_{p}"
                    )
                    src3 = cc_out[t][p].rearrange("(i pp) q -> pp i q", pp=P)
                    dst3 = cg[:].rearrange("pp (i q) -> pp i q", q=QTS)
                    nc.sync.dma_start(dst3, src3)
                    ctxg.append(cg)
                return ctxg

            def outproj_steps(t, ctxg, i0_list=(0, 2, 4, 6), op_state=None):
                # generator of small out-proj work units (2 matmuls each)
                state = op_state if op_state is not None else {}

                def unit(qs, i0):
                    if qs not in state:
                        state[qs] = ps_out.tile(
                            [P, DL], fp32, tag="out", name=f"op{t}_{qs}"
                        )
                    op = state[qs]
                    for i in (i0, i0 + 1):
                        dc = DCS[i]
                        src = ctxg[dc % 2][
                            :,
                            QTS * (dc // 2) + P * qs : QTS * (dc // 2)
                            + P * (qs + 1),
                        ]
                        nc.tensor.matmul(
                            op[:],
                            src,
                            wo_sb[dc][:],
                            start=(i == 0),
                            stop=(i == MC - 1),
                        )
                    if i0 + 2 == MC:
                        ys = y_pool.tile(
                            [P, DL], fp32, tag="ysb", name=f"ys{t}_{qs}"
                        )
                        nc.vector.tensor_copy(ys[:], op[:])
                        r = QTS * t + P * qs
                        nc.sync.dma_start(y[r : r + P, :], ys[:])

                for qs in range(4):
                    for i0 in i0_list:
                        yield lambda qs=qs, i0=i0: unit(qs, i0)

            # ---- flat slot pipeline over (u, grp) ---------------------------
            ATD = 8
            at_store = {}
            cp_store = {}
            rolling_cols = ATD * QTS

            def emit_scores(u, grp):
                pair, t = U_LIST[u]
                if grp == 0:
                    at_store[u] = {
                        h01: attn_pool.tile(
                            [P, rolling_cols], bf16, tag="attn",
                            name=f"at{u}_{h01}",
                        )
                        for h01 in range(2)
                    }
                at = at_store[u]
                mt = mts[t]
                sp = {}
                for h01 in range(2):
                    sp[h01] = ps_big.tile(
                        [P, 1024], fp32, tag="big", name=f"sp{u}_{grp}_{h01}"
                    )
                for j in range(2):
                    kc = 2 * grp + j
                    for h01 in range(2):
                        r0 = DK * h01
                        nc.tensor.matmul(
                            sp[h01][:, QTS * j : QTS * (j + 1)],
                            KT_sb[pair][r0 : r0 + DK, P * kc : P * (kc + 1)],
                            QT_sb[pair][r0 : r0 + DK, QTS * t : QTS * (t + 1)],
                            start=True,
                            stop=True,
                            tile_position=(r0, 0),
                        )
                roff = (2 * grp % ATD) * QTS
                rsl = slice(roff, roff + 1024)
                gsl = slice(1024 * grp, 1024 * (grp + 1))
                for h01 in range(2):
                    nc.scalar.activation(
                        at[h01][:, rsl],
                        sp[h01][:],
                        mybir.ActivationFunctionType.Exp,
                    )
                    nc.vector.tensor_mul(at[h01][:, rsl], at[h01][:, rsl], mt[:, gsl])

            def emit_ctx(u, grp):
                pair, t = U_LIST[u]
                if grp == 0:
                    cp_store[u] = {
                        h01: ps_acc.tile(
                            [P, QTS], fp32, tag="acc", name=f"cp{u}_{h01}"
                        )
                        for h01 in range(2)
                    }
                at = at_store[u]
                cp = cp_store[u]
                for j in range(2):
                    kc = 2 * grp + j
                    roff = (kc % ATD) * QTS
                    for h01 in range(2):
                        h = 2 * pair + h01
                        nc.tensor.matmul(
                            cp[h01][0 : DK + 1, :],
                            vext[kc][:, 65 * h : 65 * h + DK + 1],
                            at[h01][:, roff : roff + QTS],
                            start=(kc == 0),
                            stop=(kc == KC - 1),
                        )

            op_steps = []

            def emit_norm(u):
                pair, t = U_LIST[u]
                cp = cp_store[u]
                for h01 in range(2):
                    srow = sm_pool.tile(
                        [DK + 1, QTS], bf16, tag="srow", name=f"srow{u}_{h01}"
                    )
                    nc.vector.tensor_copy(
                        srow[DK : DK + 1, :], cp[h01][DK : DK + 1, :]
                    )
                    bc = ps_out.tile(
                        [P, QTS], fp32, tag="out", name=f"bc{u}_{h01}"
                    )
                    nc.tensor.matmul(
                        bc[:],
                        ones_lhs[DK : DK + 1, :],
                        srow[DK : DK + 1, :],
                        start=True,
                        stop=True,
                        tile_position=(DK, 0),
                    )
                    recipb = sm_pool.tile(
                        [P, QTS], fp32, tag="recipb", name=f"recipb{u}_{h01}"
                    )
                    nc.vector.reciprocal_approx_fast(out=recipb[:], in_=bc[:])
                    cn = ctxn_pool.tile(
                        [DK, QTS], bf16, tag="ctxn", name=f"cn{u}_{h01}"
                    )
                    nc.vector.tensor_mul(
                        cn[:], cp[h01][0:DK, :], recipb[0:DK, :]
                    )
                    nc.sync.dma_start(
                        cc_in[t][pair][DK * h01 : DK * (h01 + 1), :], cn[:]
                    )
                nc.gpsimd.collective_compute(
                    "AllGather",
                    mybir.AluOpType.bypass,
                    replica_groups=GROUPS,
                    ins=[cc_in[t][pair][:]],
                    outs=[cc_out[t][pair][:]],
                )
                del cp_store[u], at_store[u]

            NSLOT = SLOTS
            ctx_done = 0  # flat index of next ctx slot to emit

            def emit_ctx_flat(lag):
                ul, gl = divmod(lag, 8)
                emit_ctx(ul, gl)
                if gl == 7:
                    emit_norm(ul)
                    pl, tl = U_LIST[ul]
                    if pl == 1 and tl < QT_N - 1:
                        ctxg_t = do_readback(tl)
                        op_steps.extend(outproj_steps(tl, ctxg_t))

            for i in range(NSLOT):
                u, grp = divmod(i, 8)
                # trailing ctx (always-ready PE work) first, then scores,
                # then outproj in atomic 4-unit groups (a full psum
                # accumulation each -- partial groups would let a later
                # psum-ring allocation wait on not-yet-emitted matmuls),
                # then projection filler.
                target = i - 3 if i < NSLOT - 8 else i - 1
                while ctx_done <= target and ctx_done < NSLOT:
                    emit_ctx_flat(ctx_done)
                    ctx_done += 1
                emit_scores(u, grp)
                for _ in range(min(4, len(op_steps))):
                    op_steps.pop(0)()
                for fn in filler[i]:
                    fn()
            while ctx_done < NSLOT:
                emit_ctx_flat(ctx_done)
                ctx_done += 1
            for fns in filler[NSLOT]:
                fns()
            ctxg3 = do_readback(QT_N - 1)
            for st_ in op_steps:
                st_()
            for st_ in outproj_steps(QT_N - 1, ctxg3):
                st_()

    nc.compile()
    return nc


def _get_nc():
    if "nc" not in _cached:
        _cached["nc"] = _build()
    return _cached["nc"]


def _shard_inputs(q, k, v, mask, w_q, w_k, w_v, w_o):
    in_maps = []
    scale = 1.0 / np.sqrt(DK)
    wqT = (w_q.astype(np.float64) * scale).astype(np.float32).T  # [DM, DM]
    wkT = w_k.T
    wvT = w_v.T
    woT = w_o.T
    mask = np.asarray(mask)
    for c in range(8):
        b, g = c // 4, c % 4
        sl = slice(DL * g, DL * (g + 1))
        # mask pre-tiling: maskt[128t+p, 512kc... ] see kernel docstring
        mT = np.ascontiguousarray(mask[b].T).astype(BF16)  # [k, q]
        mtiled = np.ascontiguousarray(
            mT.reshape(KC, P, QT_N, QTS).transpose(2, 1, 0, 3)
        ).reshape(QT_N * P, KC * QTS)
        in_maps.append(
            {
                "qT": np.ascontiguousarray(q[b].T).astype(BF16),
                "kT": np.ascontiguousarray(k[b].T).astype(BF16),
                "vT": np.ascontiguousarray(v[b].T).astype(BF16),
                "maskt": mtiled,
                "wq": np.ascontiguousarray(wqT[:, sl]).astype(BF16),
                "wk": np.ascontiguousarray(wkT[:, sl]).astype(BF16),
                "wv": np.ascontiguousarray(wvT[:, sl]).astype(BF16),
                "wo": np.ascontiguousarray(woT[:, sl]).astype(BF16),
            }
        )
    return in_maps


def kernel(q, k, v, mask, w_q, w_k, w_v, w_o, _trace=False, _tmpdir=None):
    from concourse import bass_utils

    nc = _get_nc()
    in_maps = _shard_inputs(q, k, v, mask, w_q, w_k, w_v, w_o)
    res = bass_utils.run_bass_kernel_spmd(
        nc,
        in_maps,
        core_ids=list(range(8)),
        trace=_trace,
        tmpdir=_tmpdir,
    )
    out = np.empty((B, S, DM), dtype=np.float32)
    for c in range(8):
        b, g = c // 4, c % 4
        out[b, :, DL * g : DL * (g + 1)] = res.results[c]["y"]
    if _trace:
        _cached["last_exec_time_ns"] = res.exec_time_ns
        _cached["last_results"] = res
    return out
